# revision 52
# baseline (speedup 1.0000x reference)
"""Trainium2 Bass kernel for nn_BlockDiagonalLinearAlignment.

Math: y = x @ A, where A is a 128x128 block-diagonal matrix assembled from
dense / diagonal / low-rank 16x16 blocks, followed by row-wise L2
normalization: out = y / (||y||_2 + 1e-8).

Strategy (pure data parallel over the batch axis, 8 cores):
  - rel-err budget is 2e-2 -> compute in bf16 (measured rel err ~2.9e-3).
    Host casts x to bf16 AND pre-transposes each core shard to
    feature-major xT [128, 32768], so the kernel needs no PE transpose and
    input HBM traffic halves (16.8MB/core total vs 32MB in f32).
  - per half-chunk (16 tiles of 128 rows): PE matmuls (bf16, FWL) put y
    row-major into PSUM f32; ACT copies y to SBUF f32, freeing PSUM fast
    so the PE streams; a CUSTOM DVE op (SQSCAN_ANT, registered at import:
    inclusive prefix sum of squares) fuses square+reduce into one 1x DVE
    pass; per-tile n2 comes from differencing tile-boundary prefixes on
    DVE (same queue as the scan - no cross-engine hop); a single
    directly-emitted ACT Rsqrt gives rnorm per half
    (short dependency chains = deep pipelining); the scale out = y*rnorm is
    split DVE (SGT tiles, f32+f32-broadcast clean 1x path) / GPSIMD
    (GT-SGT tiles) to balance the two engines (~75% busy each).
  - out DMA per chunk in (partition, tile, feat) layout, bf16; host
    reorders back to row-major and upcasts to f32.
  - journey: 127.4us (f32 baseline) -> 112.5 (bf16+host transpose) ->
    99.9 (custom scan op + split scale) -> 89.0 (per-half norm chains) ->
    82.5 (3-way DVE/ACT/GPSIMD scale split, padded-prefix diffs, 2-bank
    PSUM sub-groups) -> 78.9 (CHUNK=2048: 16 finer chunks) -> 78.8
    (fused ACT Rsqrt replacing sqrt+reciprocal) -> 77.6us (prefix diffs
    moved GPSIMD->DVE, removing a cross-engine hop from the chain).
    Known residual: DVE/ACT/GPSIMD all sit at ~75-90% busy with ~1us/chunk
    of dependency stall; every +/-1-tile rebalance or granularity change
    measured 79-90us, so this split is a local optimum. DVE tensor ops
    with broadcast operands only run clean when all free dims are large
    and in0 is f32 (bf16-step1 in0 + stride-0 in1 hits a ~12 cyc/elem
    fallback uop).
"""

import contextlib
import functools
import sys

for _p in ("/opt/trn_rl_repo",):
    if _p not in sys.path:
        sys.path.append(_p)

import numpy as np
import ml_dtypes

import concourse.bacc as bacc
import concourse.bass as bass
import concourse.tile as tile
from concourse import bass_utils, mybir


def _register_sqscan():
    """Register a custom DVE op: out[p, k] = sum_{j<=k} in0[p, j]^2
    (inclusive prefix sum of squares along the free dim). Per-tile sums of
    squares are then recovered by differencing at tile boundaries, fusing
    what would otherwise be a tensor_tensor square + a tensor_reduce into
    one 1x DVE pass."""
    import re
    from concourse import dve_ops
    from concourse.dve_spec import Spec, Src0, C0, sq, scan, AluOp
    from concourse.dve_table_gen import dve_ver_for

    name = "SQSCAN2_ANT"
    for op in dve_ops.OPS:
        if op.name == name:
            return op
    spec = Spec(body=scan(AluOp.ADD, sq(Src0), init=C0))
    ver = dve_ver_for("TRN2")
    op = dve_ops.DveOp(name, spec, subdim=False, uops_sha={})
    dve_ops.OPS.append(op)
    dve_ops.CUSTOM_DVE_SPECS[name] = spec
    dve_ops._SUB_OPCODE_FOR_NAME[name] = (
        dve_ops._CUSTOM_DVE_ROW_BASE + len(dve_ops.OPS) - 1
    )
    try:
        op.compile(ver)
    except ValueError as e:
        m = re.search(r'="([0-9a-f]+)"', str(e))
        if m is None:
            raise
        op = dve_ops.DveOp(name, spec, subdim=False,
                           uops_sha={ver: m.group(1)})
        dve_ops.OPS[-1] = op
        dve_ops.CUSTOM_DVE_SPECS[name] = spec
    return op


SQSCAN = _register_sqscan()

B = 262144
D = 128
BS = 16
K = 8
N_CORES = 8
ROWS_PER_CORE = B // N_CORES  # 32768

DENSE = (0, 3, 6)
DIAG = (1, 4, 7)
LR = (2, 5)

F32 = mybir.dt.float32
BF16 = mybir.dt.bfloat16
NP_BF16 = ml_dtypes.bfloat16

P = 128
CHUNK = 2048            # rows per DMA chunk (per core)
GT = 16                 # tiles per PSUM group / half-chunk
SGT = 6                 # tiles per half scaled on DVE
AGT = 3                 # tiles per half scaled on ACT; rest go to GPSIMD
BUFS = dict(inpool=6, outpool=6, ypool=9, pfpool=7, smalls=24, ps=4)
MULT = mybir.AluOpType.mult
ADD = mybir.AluOpType.add


def _assemble_A(W_dense, s_diag, U, V):
    """Full 128x128 block-diagonal transform, y = x @ A."""
    A = np.zeros((D, D), dtype=np.float32)
    for i, k in enumerate(DENSE):
        A[k * BS:(k + 1) * BS, k * BS:(k + 1) * BS] = W_dense[i].T
    for i, k in enumerate(DIAG):
        A[k * BS:(k + 1) * BS, k * BS:(k + 1) * BS] = np.diag(s_diag[i])
    for i, k in enumerate(LR):
        A[k * BS:(k + 1) * BS, k * BS:(k + 1) * BS] = V[i] @ U[i].T
    return A


def _scalar_rsqrt(nc, out, in_):
    """ACT Rsqrt, emitted directly (bass's activation() refuses Rsqrt for
    accuracy reasons; at a 2e-2 rel-err budget and n2 in [~50, 250] the
    table accuracy is more than sufficient). Mirrors activation() lowering:
    ins = [in_, bias(AP), scale(imm), alpha(imm)]."""
    se = nc.scalar
    bias_ap = nc.const_aps.scalar_like(0.0, in_)
    ins = [
        se.lower_ap(in_),
        se.lower_ap(bias_ap),
        mybir.ImmediateValue(dtype=mybir.dt.float32, value=1.0),
        mybir.ImmediateValue(dtype=mybir.dt.float32, value=0.0),
    ]
    return se.add_instruction(
        mybir.InstActivation(
            name=nc.get_next_instruction_name(),
            func=mybir.ActivationFunctionType.Rsqrt,
            ins=ins,
            outs=[se.lower_ap(out)],
        )
    )


def _kernel_body(ctx, tc, out_ap, xT_ap, amat_ap, rows, chunk):
    nc = tc.nc
    T = chunk // P                 # tiles per chunk
    H = T // GT                    # PSUM groups (halves) per chunk
    nchunks = rows // chunk
    assert T % GT == 0 and rows % chunk == 0

    consts = ctx.enter_context(tc.tile_pool(name="consts", bufs=1))
    amat = consts.tile([P, P], BF16)
    nc.sync.dma_start(out=amat, in_=amat_ap)

    inpool = ctx.enter_context(tc.tile_pool(name="inpool", bufs=BUFS["inpool"]))
    outpool = ctx.enter_context(tc.tile_pool(name="outpool", bufs=BUFS["outpool"]))
    ypool = ctx.enter_context(tc.tile_pool(name="ypool", bufs=BUFS["ypool"]))
    pfpool = ctx.enter_context(tc.tile_pool(name="pfpool", bufs=BUFS["pfpool"]))
    smalls = ctx.enter_context(tc.tile_pool(name="smalls", bufs=BUFS["smalls"]))
    ps = ctx.enter_context(tc.tile_pool(name="ps", bufs=BUFS["ps"], space="PSUM"))

    for c in range(nchunks):
        in_sb = inpool.tile([P, chunk], BF16)
        nc.sync.dma_start(out=in_sb, in_=xT_ap[:, c * chunk:(c + 1) * chunk])
        out_sb = outpool.tile([P, T, D], BF16)

        n2 = smalls.tile([P, H, GT], F32)
        ysbs = []
        for h in range(H):
            y_sb = ypool.tile([P, GT, D], F32)
            # two 2-bank PSUM sub-groups per half: PSUM frees at 8-tile
            # granularity so the PE overlaps the ACT copies
            for q in range(2):
                y_ps = ps.tile([P, GT // 2, D], F32)
                for t in range(GT // 2):
                    g = h * GT + q * (GT // 2) + t
                    nc.tensor.matmul(
                        y_ps[:, t], lhsT=in_sb[:, g * P:(g + 1) * P],
                        rhs=amat, start=True, stop=True,
                    )
                nc.scalar.copy(y_sb[:, q * (GT // 2):(q + 1) * (GT // 2)], y_ps)
            # prefix sums of squares, with a zero pad so per-tile sums are
            # one strided subtract of tile-boundary prefixes
            pf = pfpool.tile([P, GT * D + 1], F32)
            nc.gpsimd.memset(pf[:, 0:1], 0.0)
            # seeded scan pair: scan0 runs as soon as copy0 lands and
            # overlaps copy1 on ACT; scan1 is seeded with scan0's last prefix
            HTD = (GT // 2) * D
            nc.vector._custom_dve(
                SQSCAN, out=pf[:, 1:1 + HTD], s0=0.0,
                in0=y_sb[:, 0:GT // 2].rearrange("p g d -> p (g d)"),
            )
            nc.vector._custom_dve(
                SQSCAN, out=pf[:, 1 + HTD:], s0=pf[:, HTD:HTD + 1],
                in0=y_sb[:, GT // 2:GT].rearrange("p g d -> p (g d)"),
            )
            A_hi = pf[:, 1:].rearrange("p (g d) -> p g d", d=D)[:, :, D - 1]
            A_lo = pf[:, 0:GT * D].rearrange("p (g d) -> p g d", d=D)[:, :, 0]
            nc.vector.tensor_sub(n2[:, h], A_hi, A_lo)

            rp = smalls.tile([P, GT], F32)
            _scalar_rsqrt(nc, rp, n2[:, h])
            rb = rp.unsqueeze(2).broadcast_to([P, GT, D])
            if SGT > 0:
                # f32 in0 + f32 bcast in1 is the clean 1x path on DVE
                nc.vector.tensor_mul(
                    out_sb[:, h * GT:h * GT + SGT, :],
                    y_sb[:, 0:SGT], rb[:, 0:SGT],
                )
            for t in range(SGT, SGT + AGT):     # ACT takes a few tiles
                nc.scalar.activation(
                    out_sb[:, h * GT + t, :], y_sb[:, t, :],
                    mybir.ActivationFunctionType.Copy, scale=rp[:, t:t + 1],
                )
            if SGT + AGT < GT:
                nc.gpsimd.tensor_mul(
                    out_sb[:, h * GT + SGT + AGT:(h + 1) * GT, :],
                    y_sb[:, SGT + AGT:GT], rb[:, SGT + AGT:GT],
                )

        nc.sync.dma_start(out=out_ap[c], in_=out_sb)


@functools.lru_cache(maxsize=4)
def _build(rows, chunk):
    nc = bacc.Bacc(
        "TRN2",
        target_bir_lowering=False,
        debug=False,
        num_devices=1,
    )
    nchunks = rows // chunk
    T = chunk // P
    xT_t = nc.dram_tensor("xT", [P, rows], BF16, kind="ExternalInput").ap()
    a_t = nc.dram_tensor("amat", [D, D], BF16, kind="ExternalInput").ap()
    o_t = nc.dram_tensor("out", [nchunks, P, T * D], BF16,
                         kind="ExternalOutput").ap()
    with tile.TileContext(nc) as tc, contextlib.ExitStack() as ctx:
        _kernel_body(ctx, tc, o_t, xT_t, a_t, rows, chunk)
    nc.compile()
    return nc


def _run(x, A, trace=False, trace_cores=None):
    nc = _build(ROWS_PER_CORE, CHUNK)
    # host-side shard prep: per core, feature-major bf16 [128, ROWS_PER_CORE]
    xs = x.reshape(N_CORES, ROWS_PER_CORE, D).astype(NP_BF16)
    xTs = [np.ascontiguousarray(xs[i].T) for i in range(N_CORES)]
    A16 = A.astype(NP_BF16)
    in_maps = [{"xT": xTs[i], "amat": A16} for i in range(N_CORES)]
    res = bass_utils.run_bass_kernel_spmd(
        nc, in_maps, core_ids=list(range(N_CORES)),
        trace=trace, trace_cores=trace_cores,
    )
    nchunks = ROWS_PER_CORE // CHUNK
    T = CHUNK // P
    outs = []
    for r in res.results:
        o = np.asarray(r["out"])  # [nchunks, P, T*D] bf16
        o = o.reshape(nchunks, P, T, D).transpose(0, 2, 1, 3)
        outs.append(o.reshape(ROWS_PER_CORE, D))
    out = np.concatenate(outs, axis=0).astype(np.float32)
    return out, res


def kernel(x, W_dense, s_diag, U, V):
    A = _assemble_A(
        np.asarray(W_dense, dtype=np.float32),
        np.asarray(s_diag, dtype=np.float32),
        np.asarray(U, dtype=np.float32),
        np.asarray(V, dtype=np.float32),
    )
    out, _ = _run(np.asarray(x, dtype=np.float32), A)
    return out


# revision 54
# speedup vs baseline: 1.0548x; 1.0548x over previous
"""Trainium2 Bass kernel for nn_BlockDiagonalLinearAlignment.

Math: y = x @ A, where A is a 128x128 block-diagonal matrix assembled from
dense / diagonal / low-rank 16x16 blocks, followed by row-wise L2
normalization: out = y / (||y||_2 + 1e-8).

Strategy (pure data parallel over the batch axis, 8 cores):
  - rel-err budget is 2e-2 -> compute in bf16 (measured rel err ~2.9e-3).
    Host casts x to bf16 AND pre-transposes each core shard to
    feature-major xT [128, 32768], so the kernel needs no PE transpose and
    input HBM traffic halves (16.8MB/core total vs 32MB in f32).
  - per half-chunk (16 tiles of 128 rows): PE matmuls (bf16, FWL) put y
    row-major into PSUM f32; ACT copies y to SBUF f32, freeing PSUM fast
    so the PE streams; a CUSTOM DVE op (SQSCAN_ANT, registered at import:
    inclusive prefix sum of squares) fuses square+reduce into one 1x DVE
    pass; per-tile n2 comes from differencing tile-boundary prefixes on
    DVE (same queue as the scan - no cross-engine hop); a single
    directly-emitted ACT Rsqrt gives rnorm per half
    (short dependency chains = deep pipelining); the scale out = y*rnorm is
    split DVE (SGT tiles, f32+f32-broadcast clean 1x path) / GPSIMD
    (GT-SGT tiles) to balance the two engines (~75% busy each).
  - out DMA per chunk in (partition, tile, feat) layout, bf16; host
    reorders back to row-major and upcasts to f32.
  - journey: 127.4us (f32 baseline) -> 112.5 (bf16+host transpose) ->
    99.9 (custom scan op + split scale) -> 89.0 (per-half norm chains) ->
    82.5 (3-way DVE/ACT/GPSIMD scale split, padded-prefix diffs, 2-bank
    PSUM sub-groups) -> 78.9 (CHUNK=2048: 16 finer chunks) -> 78.8
    (fused ACT Rsqrt replacing sqrt+reciprocal) -> 77.6us (prefix diffs
    moved GPSIMD->DVE, removing a cross-engine hop from the chain).
    Known residual: DVE/ACT/GPSIMD all sit at ~75-90% busy with ~1us/chunk
    of dependency stall; every +/-1-tile rebalance or granularity change
    measured 79-90us, so this split is a local optimum. DVE tensor ops
    with broadcast operands only run clean when all free dims are large
    and in0 is f32 (bf16-step1 in0 + stride-0 in1 hits a ~12 cyc/elem
    fallback uop).
"""

import contextlib
import functools
import sys

for _p in ("/opt/trn_rl_repo",):
    if _p not in sys.path:
        sys.path.append(_p)

import numpy as np
import ml_dtypes

import concourse.bacc as bacc
import concourse.bass as bass
import concourse.tile as tile
from concourse import bass_utils, mybir


def _register_sqscan():
    """Register a custom DVE op: out[p, k] = sum_{j<=k} in0[p, j]^2
    (inclusive prefix sum of squares along the free dim). Per-tile sums of
    squares are then recovered by differencing at tile boundaries, fusing
    what would otherwise be a tensor_tensor square + a tensor_reduce into
    one 1x DVE pass."""
    import re
    from concourse import dve_ops
    from concourse.dve_spec import Spec, Src0, sq, scan, AluOp
    from concourse.dve_table_gen import dve_ver_for

    name = "SQSCAN_ANT"
    for op in dve_ops.OPS:
        if op.name == name:
            return op
    spec = Spec(body=scan(AluOp.ADD, sq(Src0)))
    ver = dve_ver_for("TRN2")
    op = dve_ops.DveOp(name, spec, subdim=False, uops_sha={})
    dve_ops.OPS.append(op)
    dve_ops.CUSTOM_DVE_SPECS[name] = spec
    dve_ops._SUB_OPCODE_FOR_NAME[name] = (
        dve_ops._CUSTOM_DVE_ROW_BASE + len(dve_ops.OPS) - 1
    )
    try:
        op.compile(ver)
    except ValueError as e:
        m = re.search(r'="([0-9a-f]+)"', str(e))
        if m is None:
            raise
        op = dve_ops.DveOp(name, spec, subdim=False,
                           uops_sha={ver: m.group(1)})
        dve_ops.OPS[-1] = op
        dve_ops.CUSTOM_DVE_SPECS[name] = spec
    return op


SQSCAN = _register_sqscan()

B = 262144
D = 128
BS = 16
K = 8
N_CORES = 8
ROWS_PER_CORE = B // N_CORES  # 32768

DENSE = (0, 3, 6)
DIAG = (1, 4, 7)
LR = (2, 5)

F32 = mybir.dt.float32
BF16 = mybir.dt.bfloat16
NP_BF16 = ml_dtypes.bfloat16

P = 128
CHUNK = 2048            # rows per DMA chunk (per core)
GT = 16                 # tiles per PSUM group / half-chunk
SGT = 6                 # tiles per half scaled on DVE
AGT = 2                 # tiles per half scaled on ACT; rest go to GPSIMD
BUFS = dict(inpool=6, outpool=6, ypool=9, pfpool=7, smalls=24, ps=4)
MULT = mybir.AluOpType.mult
ADD = mybir.AluOpType.add


def _assemble_A(W_dense, s_diag, U, V):
    """Full 128x128 block-diagonal transform, y = x @ A."""
    A = np.zeros((D, D), dtype=np.float32)
    for i, k in enumerate(DENSE):
        A[k * BS:(k + 1) * BS, k * BS:(k + 1) * BS] = W_dense[i].T
    for i, k in enumerate(DIAG):
        A[k * BS:(k + 1) * BS, k * BS:(k + 1) * BS] = np.diag(s_diag[i])
    for i, k in enumerate(LR):
        A[k * BS:(k + 1) * BS, k * BS:(k + 1) * BS] = V[i] @ U[i].T
    return A


def _scalar_rsqrt(nc, out, in_):
    """ACT Rsqrt, emitted directly (bass's activation() refuses Rsqrt for
    accuracy reasons; at a 2e-2 rel-err budget and n2 in [~50, 250] the
    table accuracy is more than sufficient). Mirrors activation() lowering:
    ins = [in_, bias(AP), scale(imm), alpha(imm)]."""
    se = nc.scalar
    bias_ap = nc.const_aps.scalar_like(0.0, in_)
    ins = [
        se.lower_ap(in_),
        se.lower_ap(bias_ap),
        mybir.ImmediateValue(dtype=mybir.dt.float32, value=1.0),
        mybir.ImmediateValue(dtype=mybir.dt.float32, value=0.0),
    ]
    return se.add_instruction(
        mybir.InstActivation(
            name=nc.get_next_instruction_name(),
            func=mybir.ActivationFunctionType.Rsqrt,
            ins=ins,
            outs=[se.lower_ap(out)],
        )
    )


def _kernel_body(ctx, tc, out_ap, xT_ap, amat_ap, rows, chunk):
    nc = tc.nc
    T = chunk // P                 # tiles per chunk
    H = T // GT                    # PSUM groups (halves) per chunk
    nchunks = rows // chunk
    assert T % GT == 0 and rows % chunk == 0

    consts = ctx.enter_context(tc.tile_pool(name="consts", bufs=1))
    amat = consts.tile([P, P], BF16)
    nc.sync.dma_start(out=amat, in_=amat_ap)

    inpool = ctx.enter_context(tc.tile_pool(name="inpool", bufs=BUFS["inpool"]))
    outpool = ctx.enter_context(tc.tile_pool(name="outpool", bufs=BUFS["outpool"]))
    ypool = ctx.enter_context(tc.tile_pool(name="ypool", bufs=BUFS["ypool"]))
    pfpool = ctx.enter_context(tc.tile_pool(name="pfpool", bufs=BUFS["pfpool"]))
    smalls = ctx.enter_context(tc.tile_pool(name="smalls", bufs=BUFS["smalls"]))
    ps = ctx.enter_context(tc.tile_pool(name="ps", bufs=BUFS["ps"], space="PSUM"))

    for c in range(nchunks):
        in_sb = inpool.tile([P, chunk], BF16)
        nc.sync.dma_start(out=in_sb, in_=xT_ap[:, c * chunk:(c + 1) * chunk])
        out_sb = outpool.tile([P, T, D], BF16)

        n2 = smalls.tile([P, H, GT], F32)
        ysbs = []
        for h in range(H):
            y_sb = ypool.tile([P, GT, D], F32)
            # two 2-bank PSUM sub-groups per half: PSUM frees at 8-tile
            # granularity so the PE overlaps the ACT copies
            for q in range(2):
                y_ps = ps.tile([P, GT // 2, D], F32)
                for t in range(GT // 2):
                    g = h * GT + q * (GT // 2) + t
                    nc.tensor.matmul(
                        y_ps[:, t], lhsT=in_sb[:, g * P:(g + 1) * P],
                        rhs=amat, start=True, stop=True,
                    )
                nc.scalar.copy(y_sb[:, q * (GT // 2):(q + 1) * (GT // 2)], y_ps)
            # prefix sums of squares, with a zero pad so per-tile sums are
            # one strided subtract of tile-boundary prefixes
            pf = pfpool.tile([P, GT * D + 1], F32)
            nc.gpsimd.memset(pf[:, 0:1], 0.0)
            nc.vector._custom_dve(
                SQSCAN, out=pf[:, 1:], in0=y_sb.rearrange("p g d -> p (g d)"),
            )
            A_hi = pf[:, 1:].rearrange("p (g d) -> p g d", d=D)[:, :, D - 1]
            A_lo = pf[:, 0:GT * D].rearrange("p (g d) -> p g d", d=D)[:, :, 0]
            nc.vector.tensor_sub(n2[:, h], A_hi, A_lo)

            rp = smalls.tile([P, GT], F32)
            _scalar_rsqrt(nc, rp, n2[:, h])
            rb = rp.unsqueeze(2).broadcast_to([P, GT, D])
            if SGT > 0:
                # f32 in0 + f32 bcast in1 is the clean 1x path on DVE
                nc.vector.tensor_mul(
                    out_sb[:, h * GT:h * GT + SGT, :],
                    y_sb[:, 0:SGT], rb[:, 0:SGT],
                )
            for t in range(SGT, SGT + AGT):     # ACT takes a few tiles
                nc.scalar.activation(
                    out_sb[:, h * GT + t, :], y_sb[:, t, :],
                    mybir.ActivationFunctionType.Copy, scale=rp[:, t:t + 1],
                )
            if SGT + AGT < GT:
                nc.gpsimd.tensor_mul(
                    out_sb[:, h * GT + SGT + AGT:(h + 1) * GT, :],
                    y_sb[:, SGT + AGT:GT], rb[:, SGT + AGT:GT],
                )

        nc.sync.dma_start(out=out_ap[c], in_=out_sb)


@functools.lru_cache(maxsize=4)
def _build(rows, chunk):
    nc = bacc.Bacc(
        "TRN2",
        target_bir_lowering=False,
        debug=False,
        num_devices=1,
    )
    nchunks = rows // chunk
    T = chunk // P
    xT_t = nc.dram_tensor("xT", [P, rows], BF16, kind="ExternalInput").ap()
    a_t = nc.dram_tensor("amat", [D, D], BF16, kind="ExternalInput").ap()
    o_t = nc.dram_tensor("out", [nchunks, P, T * D], BF16,
                         kind="ExternalOutput").ap()
    with tile.TileContext(nc) as tc, contextlib.ExitStack() as ctx:
        _kernel_body(ctx, tc, o_t, xT_t, a_t, rows, chunk)
    nc.compile()
    return nc


def _run(x, A, trace=False, trace_cores=None):
    nc = _build(ROWS_PER_CORE, CHUNK)
    # host-side shard prep: per core, feature-major bf16 [128, ROWS_PER_CORE]
    xs = x.reshape(N_CORES, ROWS_PER_CORE, D).astype(NP_BF16)
    xTs = [np.ascontiguousarray(xs[i].T) for i in range(N_CORES)]
    A16 = A.astype(NP_BF16)
    in_maps = [{"xT": xTs[i], "amat": A16} for i in range(N_CORES)]
    res = bass_utils.run_bass_kernel_spmd(
        nc, in_maps, core_ids=list(range(N_CORES)),
        trace=trace, trace_cores=trace_cores,
    )
    nchunks = ROWS_PER_CORE // CHUNK
    T = CHUNK // P
    outs = []
    for r in res.results:
        o = np.asarray(r["out"])  # [nchunks, P, T*D] bf16
        o = o.reshape(nchunks, P, T, D).transpose(0, 2, 1, 3)
        outs.append(o.reshape(ROWS_PER_CORE, D))
    out = np.concatenate(outs, axis=0).astype(np.float32)
    return out, res


def kernel(x, W_dense, s_diag, U, V):
    A = _assemble_A(
        np.asarray(W_dense, dtype=np.float32),
        np.asarray(s_diag, dtype=np.float32),
        np.asarray(U, dtype=np.float32),
        np.asarray(V, dtype=np.float32),
    )
    out, _ = _run(np.asarray(x, dtype=np.float32), A)
    return out


# revision 56
# speedup vs baseline: 1.0806x; 1.0244x over previous
"""Trainium2 Bass kernel for nn_BlockDiagonalLinearAlignment.

Math: y = x @ A, where A is a 128x128 block-diagonal matrix assembled from
dense / diagonal / low-rank 16x16 blocks, followed by row-wise L2
normalization: out = y / (||y||_2 + 1e-8).

Strategy (pure data parallel over the batch axis, 8 cores):
  - rel-err budget is 2e-2 -> compute in bf16 (measured rel err ~2.9e-3).
    Host casts x to bf16 AND pre-transposes each core shard to
    feature-major xT [128, 32768], so the kernel needs no PE transpose and
    input HBM traffic halves (16.8MB/core total vs 32MB in f32).
  - per half-chunk (16 tiles of 128 rows): PE matmuls (bf16, FWL) put y
    row-major into PSUM f32; ACT copies y to SBUF f32, freeing PSUM fast
    so the PE streams; a CUSTOM DVE op (SQSCAN_ANT, registered at import:
    inclusive prefix sum of squares) fuses square+reduce into one 1x DVE
    pass; per-tile n2 comes from differencing tile-boundary prefixes on
    DVE (same queue as the scan - no cross-engine hop); a single
    directly-emitted ACT Rsqrt gives rnorm per half
    (short dependency chains = deep pipelining); the scale out = y*rnorm is
    split DVE (SGT tiles, f32+f32-broadcast clean 1x path) / GPSIMD
    (GT-SGT tiles) to balance the two engines (~75% busy each).
  - out DMA per chunk in (partition, tile, feat) layout, bf16; host
    reorders back to row-major and upcasts to f32.
  - journey: 127.4us (f32 baseline) -> 112.5 (bf16+host transpose) ->
    99.9 (custom scan op + split scale) -> 89.0 (per-half norm chains) ->
    82.5 (3-way DVE/ACT/GPSIMD scale split, padded-prefix diffs, 2-bank
    PSUM sub-groups) -> 78.9 (CHUNK=2048: 16 finer chunks) -> 78.8
    (fused ACT Rsqrt replacing sqrt+reciprocal) -> 77.6us (prefix diffs
    moved GPSIMD->DVE, removing a cross-engine hop from the chain).
    Known residual: DVE/ACT/GPSIMD all sit at ~75-90% busy with ~1us/chunk
    of dependency stall; every +/-1-tile rebalance or granularity change
    measured 79-90us, so this split is a local optimum. DVE tensor ops
    with broadcast operands only run clean when all free dims are large
    and in0 is f32 (bf16-step1 in0 + stride-0 in1 hits a ~12 cyc/elem
    fallback uop).
"""

import contextlib
import functools
import sys

for _p in ("/opt/trn_rl_repo",):
    if _p not in sys.path:
        sys.path.append(_p)

import numpy as np
import ml_dtypes

import concourse.bacc as bacc
import concourse.bass as bass
import concourse.tile as tile
from concourse import bass_utils, mybir


def _register_sqscan():
    """Register a custom DVE op: out[p, k] = sum_{j<=k} in0[p, j]^2
    (inclusive prefix sum of squares along the free dim). Per-tile sums of
    squares are then recovered by differencing at tile boundaries, fusing
    what would otherwise be a tensor_tensor square + a tensor_reduce into
    one 1x DVE pass."""
    import re
    from concourse import dve_ops
    from concourse.dve_spec import Spec, Src0, sq, scan, AluOp
    from concourse.dve_table_gen import dve_ver_for

    name = "SQSCAN_ANT"
    for op in dve_ops.OPS:
        if op.name == name:
            return op
    spec = Spec(body=scan(AluOp.ADD, sq(Src0)))
    ver = dve_ver_for("TRN2")
    op = dve_ops.DveOp(name, spec, subdim=False, uops_sha={})
    dve_ops.OPS.append(op)
    dve_ops.CUSTOM_DVE_SPECS[name] = spec
    dve_ops._SUB_OPCODE_FOR_NAME[name] = (
        dve_ops._CUSTOM_DVE_ROW_BASE + len(dve_ops.OPS) - 1
    )
    try:
        op.compile(ver)
    except ValueError as e:
        m = re.search(r'="([0-9a-f]+)"', str(e))
        if m is None:
            raise
        op = dve_ops.DveOp(name, spec, subdim=False,
                           uops_sha={ver: m.group(1)})
        dve_ops.OPS[-1] = op
        dve_ops.CUSTOM_DVE_SPECS[name] = spec
    return op


SQSCAN = _register_sqscan()

B = 262144
D = 128
BS = 16
K = 8
N_CORES = 8
ROWS_PER_CORE = B // N_CORES  # 32768

DENSE = (0, 3, 6)
DIAG = (1, 4, 7)
LR = (2, 5)

F32 = mybir.dt.float32
BF16 = mybir.dt.bfloat16
NP_BF16 = ml_dtypes.bfloat16

P = 128
CHUNK = 2048            # rows per DMA chunk (per core)
GT = 16                 # tiles per PSUM group / half-chunk
SGT = 6                 # tiles per half scaled on DVE
AGT = 3                 # tiles per half scaled on ACT; rest go to GPSIMD
BUFS = dict(inpool=6, outpool=6, ypool=9, pfpool=7, smalls=24, ps=4)
MULT = mybir.AluOpType.mult
ADD = mybir.AluOpType.add


def _assemble_A(W_dense, s_diag, U, V):
    """Full 128x128 block-diagonal transform, y = x @ A."""
    A = np.zeros((D, D), dtype=np.float32)
    for i, k in enumerate(DENSE):
        A[k * BS:(k + 1) * BS, k * BS:(k + 1) * BS] = W_dense[i].T
    for i, k in enumerate(DIAG):
        A[k * BS:(k + 1) * BS, k * BS:(k + 1) * BS] = np.diag(s_diag[i])
    for i, k in enumerate(LR):
        A[k * BS:(k + 1) * BS, k * BS:(k + 1) * BS] = V[i] @ U[i].T
    return A


def _scalar_rsqrt(nc, out, in_):
    """ACT Rsqrt, emitted directly (bass's activation() refuses Rsqrt for
    accuracy reasons; at a 2e-2 rel-err budget and n2 in [~50, 250] the
    table accuracy is more than sufficient). Mirrors activation() lowering:
    ins = [in_, bias(AP), scale(imm), alpha(imm)]."""
    se = nc.scalar
    bias_ap = nc.const_aps.scalar_like(0.0, in_)
    ins = [
        se.lower_ap(in_),
        se.lower_ap(bias_ap),
        mybir.ImmediateValue(dtype=mybir.dt.float32, value=1.0),
        mybir.ImmediateValue(dtype=mybir.dt.float32, value=0.0),
    ]
    return se.add_instruction(
        mybir.InstActivation(
            name=nc.get_next_instruction_name(),
            func=mybir.ActivationFunctionType.Rsqrt,
            ins=ins,
            outs=[se.lower_ap(out)],
        )
    )


def _kernel_body(ctx, tc, out_ap, xT_ap, amat_ap, rows, chunk):
    nc = tc.nc
    T = chunk // P                 # tiles per chunk
    H = T // GT                    # PSUM groups (halves) per chunk
    nchunks = rows // chunk
    assert T % GT == 0 and rows % chunk == 0

    consts = ctx.enter_context(tc.tile_pool(name="consts", bufs=1))
    amat = consts.tile([P, P], BF16)
    nc.sync.dma_start(out=amat, in_=amat_ap)

    inpool = ctx.enter_context(tc.tile_pool(name="inpool", bufs=BUFS["inpool"]))
    outpool = ctx.enter_context(tc.tile_pool(name="outpool", bufs=BUFS["outpool"]))
    ypool = ctx.enter_context(tc.tile_pool(name="ypool", bufs=BUFS["ypool"]))
    pfpool = ctx.enter_context(tc.tile_pool(name="pfpool", bufs=BUFS["pfpool"]))
    smalls = ctx.enter_context(tc.tile_pool(name="smalls", bufs=BUFS["smalls"]))
    ps = ctx.enter_context(tc.tile_pool(name="ps", bufs=BUFS["ps"], space="PSUM"))

    assert H == 1, "tail pipelining below assumes one PSUM group per chunk"
    pending = None

    def emit_tail(n2, y_sb, out_sb, c):
        # finish phase: rsqrt + 3-way scale + out DMA. Emitted one chunk
        # late so the next chunk's ACT copies are ahead of this chunk's
        # rsqrt in ACT's strict FIFO (avoids head-of-line blocking while
        # rsqrt waits on the DVE scan+diff chain).
        rp = smalls.tile([P, GT], F32)
        _scalar_rsqrt(nc, rp, n2[:, 0])
        rb = rp.unsqueeze(2).broadcast_to([P, GT, D])
        nc.vector.tensor_mul(
            out_sb[:, 0:SGT, :], y_sb[:, 0:SGT], rb[:, 0:SGT],
        )
        for t in range(SGT, SGT + AGT):
            nc.scalar.activation(
                out_sb[:, t, :], y_sb[:, t, :],
                mybir.ActivationFunctionType.Copy, scale=rp[:, t:t + 1],
            )
        nc.gpsimd.tensor_mul(
            out_sb[:, SGT + AGT:GT, :], y_sb[:, SGT + AGT:GT],
            rb[:, SGT + AGT:GT],
        )
        nc.sync.dma_start(out=out_ap[c], in_=out_sb)

    for c in range(nchunks):
        in_sb = inpool.tile([P, chunk], BF16)
        nc.sync.dma_start(out=in_sb, in_=xT_ap[:, c * chunk:(c + 1) * chunk])
        out_sb = outpool.tile([P, T, D], BF16)

        n2 = smalls.tile([P, H, GT], F32)
        y_sb = ypool.tile([P, GT, D], F32)
        # two 2-bank PSUM sub-groups: PSUM frees at 8-tile granularity so
        # the PE overlaps the ACT copies
        for q in range(2):
            y_ps = ps.tile([P, GT // 2, D], F32)
            for t in range(GT // 2):
                g = q * (GT // 2) + t
                nc.tensor.matmul(
                    y_ps[:, t], lhsT=in_sb[:, g * P:(g + 1) * P],
                    rhs=amat, start=True, stop=True,
                )
            nc.scalar.copy(y_sb[:, q * (GT // 2):(q + 1) * (GT // 2)], y_ps)
        # prefix sums of squares, with a zero pad so per-tile sums are
        # one strided subtract of tile-boundary prefixes
        pf = pfpool.tile([P, GT * D + 1], F32)
        nc.gpsimd.memset(pf[:, 0:1], 0.0)
        nc.vector._custom_dve(
            SQSCAN, out=pf[:, 1:], in0=y_sb.rearrange("p g d -> p (g d)"),
        )
        A_hi = pf[:, 1:].rearrange("p (g d) -> p g d", d=D)[:, :, D - 1]
        A_lo = pf[:, 0:GT * D].rearrange("p (g d) -> p g d", d=D)[:, :, 0]
        nc.vector.tensor_sub(n2[:, 0], A_hi, A_lo)

        if pending is not None:
            emit_tail(*pending)
        pending = (n2, y_sb, out_sb, c)
    emit_tail(*pending)


@functools.lru_cache(maxsize=4)
def _build(rows, chunk):
    nc = bacc.Bacc(
        "TRN2",
        target_bir_lowering=False,
        debug=False,
        num_devices=1,
    )
    nchunks = rows // chunk
    T = chunk // P
    xT_t = nc.dram_tensor("xT", [P, rows], BF16, kind="ExternalInput").ap()
    a_t = nc.dram_tensor("amat", [D, D], BF16, kind="ExternalInput").ap()
    o_t = nc.dram_tensor("out", [nchunks, P, T * D], BF16,
                         kind="ExternalOutput").ap()
    with tile.TileContext(nc) as tc, contextlib.ExitStack() as ctx:
        _kernel_body(ctx, tc, o_t, xT_t, a_t, rows, chunk)
    nc.compile()
    return nc


def _run(x, A, trace=False, trace_cores=None):
    nc = _build(ROWS_PER_CORE, CHUNK)
    # host-side shard prep: per core, feature-major bf16 [128, ROWS_PER_CORE]
    xs = x.reshape(N_CORES, ROWS_PER_CORE, D).astype(NP_BF16)
    xTs = [np.ascontiguousarray(xs[i].T) for i in range(N_CORES)]
    A16 = A.astype(NP_BF16)
    in_maps = [{"xT": xTs[i], "amat": A16} for i in range(N_CORES)]
    res = bass_utils.run_bass_kernel_spmd(
        nc, in_maps, core_ids=list(range(N_CORES)),
        trace=trace, trace_cores=trace_cores,
    )
    nchunks = ROWS_PER_CORE // CHUNK
    T = CHUNK // P
    outs = []
    for r in res.results:
        o = np.asarray(r["out"])  # [nchunks, P, T*D] bf16
        o = o.reshape(nchunks, P, T, D).transpose(0, 2, 1, 3)
        outs.append(o.reshape(ROWS_PER_CORE, D))
    out = np.concatenate(outs, axis=0).astype(np.float32)
    return out, res


def kernel(x, W_dense, s_diag, U, V):
    A = _assemble_A(
        np.asarray(W_dense, dtype=np.float32),
        np.asarray(s_diag, dtype=np.float32),
        np.asarray(U, dtype=np.float32),
        np.asarray(V, dtype=np.float32),
    )
    out, _ = _run(np.asarray(x, dtype=np.float32), A)
    return out
